# revision 1
# baseline (speedup 1.0000x reference)
"""Multi-head attention (S=4096, D=1024, H=16, dk=dv=64) on 8 trn2 NeuronCores.

Sharding: tensor-parallel over heads — 2 heads per core. Each core receives
the (host-transposed, bf16-cast) activations plus its two heads' projection
weights and its 128-column slice of Wo, computes its heads' attention and a
partial output product y_c = hc @ Wo[:, c-slice].T, and the host all-reduces
the 8 partials (the "row-shard W_o and all-reduce" variant, with the
all-reduce in the gather step).

Per-core Bass/Tile kernel (engines execute their streams in order, so the
program is software-pipelined by emission order):
  - Projections qhT/khT [128,S] (head A on partitions 0-63, head B on 64-127)
    and vh tiles [sk, dv]+ones-column (PE-transposed), streamed per 512-wide
    strip and interleaved into strip 0's attention loop (DMA-paced).
  - Attention, ACT-paced steady state: per sk tile, scores^T for both heads
    concurrently (PE row groups at base partitions 0/64), one Exp over the
    [128,1024] PSUM pair (scale=1/8; no max subtraction — scores are O(1)),
    two PV accumulations out^T[dv+1, sq]; the ones column yields the softmax
    denominator in row 64.
  - Per-strip epilogue, drip-fed into the NEXT strip's loop: one reciprocal
    over both denominator rows (adjacent partitions via an SBUF->SBUF DMA
    partition move), K=2 selector-matmul broadcast across 64 partitions
    (DVE cannot cross partitions), in-place normalize, head B shifted under
    head A by DMA, then single K=128 output-projection matmuls.

Matmul operands are bf16 (full PE rate; fp32 is 1/4 rate and fp32r is an
e11m8 format with the same 8-bit mantissa). PSUM accumulation is fp32; exp
input is exact fp32; denominators stay fp32 through the reciprocal.
PSUM (8 banks): qk pairs 2x[128,1024]=4, pv 2x[128,512]=2, post 2x[128,512]=2.
"""

import contextlib
import sys

if "/opt/trn_rl_repo" not in sys.path:
    sys.path.insert(0, "/opt/trn_rl_repo")

import numpy as np
import ml_dtypes

import concourse.bass as bass
import concourse.mybir as mybir
import concourse.tile as tile
from concourse.masks import make_identity

F32 = mybir.dt.float32
BF16 = mybir.dt.bfloat16
EXP = mybir.ActivationFunctionType.Exp
MULT = mybir.AluOpType.mult

S, D, DK, P, W = 4096, 1024, 64, 128, 512
NS = S // W      # 8 sq strips
NT = S // P      # 32 sk tiles
NDC = D // P     # 8 contraction chunks for projections
TPW = W // P     # sk tiles per strip (4)
SCALE = 0.125    # 1/sqrt(DK)
NCORES = 8


def _split_excess_waits(nc, max_waits=1, max_waits_evsem=2):
    """The walrus build in this container rejects instructions carrying more
    than ~2 sync-wait commands; Tile's exit drain aggregates one wait per live
    semaphore onto single instructions. Split the excess onto preceding NoOps
    on the same engine (engine streams are in-order, so semantics hold)."""
    for fn in nc.m.functions:
        for blk in fn.blocks:
            new_insts = []
            for inst in blk.instructions:
                si = getattr(inst, "sync_info", None)
                lim = (
                    max_waits_evsem
                    if isinstance(inst, mybir.InstEventSemaphore)
                    else max_waits
                )
                if si is not None and si.on_wait and len(si.on_wait) > lim:
                    waits = list(si.on_wait)
                    for w in waits[:-lim]:
                        new_insts.append(
                            mybir.InstNoOp(
                                name=nc.get_next_instruction_name(),
                                engine=inst.engine,
                                bass_nofuse=True,
                                sync_info=mybir.SyncInfo(on_wait=[w], on_update=[]),
                            )
                        )
                    si.on_wait = waits[-lim:]
                new_insts.append(inst)
            blk.instructions = new_insts


def _build_mha(nc: bass.Bass):
    qT = nc.dram_tensor("qT", [D, S], BF16, kind="ExternalInput")
    kT = nc.dram_tensor("kT", [D, S], BF16, kind="ExternalInput")
    vT = nc.dram_tensor("vT", [D, S], BF16, kind="ExternalInput")
    wq = nc.dram_tensor("wq", [D, P], BF16, kind="ExternalInput")
    wk = nc.dram_tensor("wk", [D, P], BF16, kind="ExternalInput")
    wv = nc.dram_tensor("wv", [D, P], BF16, kind="ExternalInput")
    wo = nc.dram_tensor("wo", [P, D], BF16, kind="ExternalInput")
    seld = nc.dram_tensor("seld", [2, 2 * DK], BF16, kind="ExternalInput")
    y = nc.dram_tensor("y", [S, D], BF16, kind="ExternalOutput")

    qT3 = qT.rearrange("(o p) s -> p o s", p=P)
    kT3 = kT.rearrange("(o p) s -> p o s", p=P)
    vT3 = vT.rearrange("(o p) s -> p o s", p=P)
    wq3 = wq.rearrange("(o p) m -> p o m", p=P)
    wk3 = wk.rearrange("(o p) m -> p o m", p=P)
    wv3 = wv.rearrange("(o p) m -> p o m", p=P)

    with tile.TileContext(nc) as tc, contextlib.ExitStack() as ctx:
        static = ctx.enter_context(tc.tile_pool(name="static", bufs=1))
        xpool = ctx.enter_context(tc.tile_pool(name="x", bufs=8))
        vtmp = ctx.enter_context(tc.tile_pool(name="vtmp", bufs=2))
        ptp = ctx.enter_context(tc.tile_pool(name="pt", bufs=6))
        recp = ctx.enter_context(tc.tile_pool(name="rec", bufs=2))
        ystage = ctx.enter_context(tc.tile_pool(name="ystage", bufs=4))
        qk_ps = ctx.enter_context(tc.tile_pool(name="qkps", bufs=2, space="PSUM"))
        pv_ps = ctx.enter_context(tc.tile_pool(name="pvps", bufs=2, space="PSUM"))
        post_ps = ctx.enter_context(tc.tile_pool(name="postps", bufs=2, space="PSUM"))

        wq_sb = static.tile([P, NDC, P], BF16, tag="wq")
        wk_sb = static.tile([P, NDC, P], BF16, tag="wk")
        wv_sb = static.tile([P, NDC, P], BF16, tag="wv")
        wo_sb = static.tile([P, D], BF16, tag="wo")
        ident = static.tile([P, P], BF16, tag="ident")
        sel = static.tile([DK + 2, 2 * DK], BF16, tag="sel")
        khT = static.tile([P, S], BF16, tag="khT")
        qhT = static.tile([P, S], BF16, tag="qhT")
        vh = static.tile([P, NT, 2 * DK + 2], BF16, tag="vh")
        hc = static.tile([P, S], BF16, tag="hc")
        tmpb_pool = recp

        nc.sync.dma_start(wq_sb[:], wq3)
        nc.sync.dma_start(wk_sb[:], wk3)
        nc.sync.dma_start(wv_sb[:], wv3)
        make_identity(nc, ident[:])
        # HAM warm-up: ~4us of dummy PE work (no DMA dependency) so the
        # projections hit the array already at 2.4 GHz.
        warm = post_ps.tile([P, W], F32, tag="post")
        for _ in range(40):
            nc.tensor.matmul(warm[:, 0:P], ident[:], ident[:], start=True, stop=True)

        def one_proj(jw, w_sb, src3, dst):
            def _th():
                xx = xpool.tile([P, NDC, W], BF16, tag="xs")
                nc.sync.dma_start(xx[:], src3[:, :, jw])
                pp = post_ps.tile([P, W], F32, tag="post")
                for c in range(NDC):
                    nc.tensor.matmul(
                        pp[:], w_sb[:, c, :], xx[:, c, :],
                        start=(c == 0), stop=(c == NDC - 1),
                    )
                nc.vector.tensor_copy(dst, pp[:])
            return _th

        def proj_thunks(j):
            """Projection strip j as 4 thunks: q, k, v, v-transposes."""
            jw = slice(j * W, (j + 1) * W)
            vts = vtmp.tile([P, W], BF16, tag="vts")

            def _trs():
                for i in range(TPW):
                    t = j * TPW + i
                    ptr = post_ps.tile([P, P], BF16, tag="post")
                    nc.tensor.transpose(ptr[:], vts[:, i * P : (i + 1) * P], ident[:])
                    # ptr rows = sk; cols 0:64 head A dv, 64:128 head B dv
                    nc.vector.tensor_copy(vh[:, t, 0:DK], ptr[:, 0:DK])
                    nc.vector.tensor_copy(
                        vh[:, t, DK + 1 : 2 * DK + 1], ptr[:, DK : 2 * DK]
                    )
                    nc.gpsimd.memset(vh[:, t, DK : DK + 1], 1.0)
                    nc.gpsimd.memset(vh[:, t, 2 * DK + 1 : 2 * DK + 2], 1.0)

            return [
                one_proj(jw, wq_sb, qT3, qhT[:, jw]),
                one_proj(jw, wk_sb, kT3, khT[:, jw]),
                one_proj(jw, wv_sb, vT3, vts[:]),
                _trs,
            ]

        def emit_proj(j):
            for th in proj_thunks(j):
                th()

        def make_epilogue(s, rec, tmpb):
            """Deferred post-softmax work for strip s, drip-fed into the next
            strip's attention loop (fills PE slack under the exp pace)."""
            cw = slice(s * W, (s + 1) * W)
            thunks = []

            def norm_a():
                bc = post_ps.tile([P, W], F32, tag="post")
                nc.tensor.matmul(
                    bc[0:DK, :], sel[DK : DK + 2, 0:DK], rec[DK : DK + 2, :],
                    start=True, stop=True,
                )
                nc.vector.tensor_tensor(
                    hc[0:DK, cw], hc[0:DK, cw], bc[0:DK, :], op=MULT
                )

            def norm_b():
                bc = post_ps.tile([P, W], F32, tag="post")
                nc.tensor.matmul(
                    bc[0:DK, :], sel[DK : DK + 2, DK : 2 * DK], rec[DK : DK + 2, :],
                    start=True, stop=True,
                )
                nc.vector.tensor_tensor(
                    tmpb[0:DK, :], tmpb[0:DK, :], bc[0:DK, :], op=MULT
                )
                nc.sync.dma_start(hc[DK:P, cw], tmpb[0:DK, :])

            thunks.append(norm_a)
            thunks.append(norm_b)

            def proj_out(i, oh):
                def _th():
                    sq = s * TPW + i
                    if s == NS - 1:
                        # tail: the qk banks are dead — 2 free slots beat
                        # WAR-serializing through the shared post pool
                        py = qk_ps.tile([P, W], F32, tag="qk")
                    else:
                        py = post_ps.tile([P, W], F32, tag="post")
                    nc.tensor.matmul(
                        py[:],
                        hc[:, sq * P : (sq + 1) * P],
                        wo_sb[:, oh * W : (oh + 1) * W],
                        start=True, stop=True,
                    )
                    ys = ystage.tile([P, W], BF16, tag="ys")
                    # final strip: exp stream is over, ScalarE is idle — let it
                    # share the PSUM evacuations so the tail chain is shorter
                    if s == NS - 1 and oh == 1:
                        nc.scalar.copy(ys[:], py[:])
                    else:
                        nc.vector.tensor_copy(ys[:], py[:])
                    nc.sync.dma_start(
                        y[sq * P : (sq + 1) * P, oh * W : (oh + 1) * W], ys[:]
                    )
                return _th

            for i in range(TPW):
                for oh in range(2):
                    thunks.append(proj_out(i, oh))
            return thunks

        # ---- main software-pipelined loop ----
        emit_proj(0)
        nc.sync.dma_start(wo_sb[:], wo[:])
        nc.sync.dma_start(sel[DK : DK + 2, :], seld[:])
        # per-strip projection thunks in dependency order; q(1) stays in
        # strip 0's drip (its consumer starts next strip), q(2..7) drip one
        # per strip s-1 instead of piling onto strip 0's PE
        projq = []
        qdrip = {}
        for j in range(1, NS):
            q_th, k_th, v_th, trs_th = proj_thunks(j)
            projq += [k_th, v_th, trs_th]
            if j == 1:
                projq.append(q_th)
            else:
                qdrip[j - 1] = q_th

        def emit_qk(s, t):
            """Scores^T for both heads of tile t against sq strip s."""
            scw = slice(s * W, (s + 1) * W)
            qk = qk_ps.tile([P, 2 * W], F32, tag="qk")
            nc.tensor.matmul(
                qk[0:P, 0:W],
                khT[0:DK, t * P : (t + 1) * P], qhT[0:DK, scw],
                start=True, stop=True,
            )
            nc.tensor.matmul(
                qk[0:P, W : 2 * W],
                khT[DK:P, t * P : (t + 1) * P], qhT[DK:P, scw],
                start=True, stop=True,
            )
            return qk

        pending = []
        qk_cur = emit_qk(0, 0)
        for s in range(NS):
            cw = slice(s * W, (s + 1) * W)
            pva = pv_ps.tile([P, W], F32, tag="pv")
            pvb = pv_ps.tile([P, W], F32, tag="pv")
            for t in range(NT):
                if s == 0 and projq:
                    projq.pop(0)()
                pt = ptp.tile([P, 2 * W], BF16, tag="pt")
                nc.scalar.activation(pt[:], qk_cur[:], EXP, scale=SCALE)
                # one-window QK lookahead: emit the NEXT tile's scores before
                # this tile's PV so the strip-boundary pipeline never refills
                if t + 1 < NT:
                    qk_cur = emit_qk(s, t + 1)
                elif s + 1 < NS:
                    qk_cur = emit_qk(s + 1, 0)
                nc.tensor.matmul(
                    pva[0 : DK + 1, :],
                    vh[:, t, 0 : DK + 1], pt[:, 0:W],
                    start=(t == 0), stop=(t == NT - 1),
                )
                nc.tensor.matmul(
                    pvb[0 : DK + 1, :],
                    vh[:, t, DK + 1 : 2 * DK + 2], pt[:, W : 2 * W],
                    start=(t == 0), stop=(t == NT - 1),
                )
                if pending and t >= 16 and t % 2 == 0:
                    pending.pop(0)()
                    if t == NT - 2:
                        while pending:
                            pending.pop(0)()

            # strip boundary: evacuate PSUM fast (head A + head B + the two
            # denominator rows, B's moved to the adjacent partition by DMA),
            # then one reciprocal for both heads — all off the exp path.
            # evacuate the big pv regions FIRST so the next strip's PV can
            # claim the banks before the slow reciprocal occupies the DVE
            rs = recp.tile([P, W], F32, tag="rs")
            tmpb = tmpb_pool.tile([DK, W], BF16, tag="tmpb")
            if s == NS - 1:
                # tail: exp stream is over — ScalarE takes the big copies in
                # parallel with the DVE's denominator/reciprocal chain
                nc.scalar.copy(hc[0:DK, cw], pva[0:DK, :])
                nc.scalar.copy(tmpb[0:DK, :], pvb[0:DK, :])
            else:
                nc.vector.tensor_copy(hc[0:DK, cw], pva[0:DK, :])
                nc.vector.tensor_copy(tmpb[0:DK, :], pvb[0:DK, :])
            nc.vector.tensor_copy(rs[DK : DK + 1, :], pva[DK : DK + 1, :])
            rbt = recp.tile([P, W], F32, tag="rbt")
            nc.vector.tensor_copy(rbt[DK : DK + 1, :], pvb[DK : DK + 1, :])
            nc.sync.dma_start(rs[DK + 1 : DK + 2, :], rbt[DK : DK + 1, :])
            rec = recp.tile([P, W], BF16, tag="rec")
            with nc.allow_low_precision(
                reason="bf16 softmax denominators feed a bf16 matmul broadcast"
            ):
                nc.vector.reciprocal(rec[DK : DK + 2, :], rs[DK : DK + 2, :])

            assert not pending
            pending = make_epilogue(s, rec, tmpb)
            if (s + 1) in qdrip:
                pending.insert(0, qdrip.pop(s + 1))

        for th in pending:
            th()
    return nc


def _make_core_inputs(q, k, v, Wq, Wk, Wv, Wo, core, cache):
    bf = ml_dtypes.bfloat16
    if "qT" not in cache:
        cache["qT"] = np.ascontiguousarray(q.T).astype(bf)
        cache["kT"] = np.ascontiguousarray(k.T).astype(bf)
        cache["vT"] = np.ascontiguousarray(v.T).astype(bf)
    h0, h1 = 2 * core, 2 * core + 1
    return {
        "qT": cache["qT"],
        "kT": cache["kT"],
        "vT": cache["vT"],
        "wq": np.concatenate([Wq[h0], Wq[h1]], axis=1).astype(bf),
        "wk": np.concatenate([Wk[h0], Wk[h1]], axis=1).astype(bf),
        "wv": np.concatenate([Wv[h0], Wv[h1]], axis=1).astype(bf),
        "wo": np.ascontiguousarray(Wo[:, P * core : P * (core + 1)].T).astype(bf),
        "seld": np.kron(
            np.eye(2, dtype=np.float32), np.ones((1, DK), np.float32)
        ).astype(bf),
    }


_NC = None
last_results = None  # BassKernelResults of the most recent run (for profiling)


def _get_nc():
    global _NC
    if _NC is None:
        nc = bass.Bass("TRN2", target_bir_lowering=False, debug=False)
        _build_mha(nc)
        _split_excess_waits(nc)
        _NC = nc
    return _NC


def kernel(q, k, v, Wq, Wk, Wv, Wo, **run_kwargs):
    """Full-input MHA forward. Shards over 8 NeuronCores (2 heads each),
    runs the Bass kernel, and all-reduces the per-core partial outputs."""
    from concourse.bass_utils import run_bass_kernel_spmd

    global last_results
    q = np.asarray(q, np.float32)
    k = np.asarray(k, np.float32)
    v = np.asarray(v, np.float32)
    Wq = np.asarray(Wq, np.float32)
    Wk = np.asarray(Wk, np.float32)
    Wv = np.asarray(Wv, np.float32)
    Wo = np.asarray(Wo, np.float32)

    nc = _get_nc()
    cache = {}
    in_maps = [
        _make_core_inputs(q, k, v, Wq, Wk, Wv, Wo, c, cache) for c in range(NCORES)
    ]
    res = run_bass_kernel_spmd(
        nc, in_maps, core_ids=list(range(NCORES)), **run_kwargs
    )
    last_results = res
    y = res.results[0]["y"].astype(np.float32)
    for c in range(1, NCORES):
        y += res.results[c]["y"]
    return y



# revision 17
# speedup vs baseline: 1.0004x; 1.0004x over previous
"""Multi-head attention (S=4096, D=1024, H=16, dk=dv=64) on 8 trn2 NeuronCores.

Sharding: tensor-parallel over heads — 2 heads per core. Each core receives
the (host-transposed, bf16-cast) activations plus its two heads' projection
weights and its 128-column slice of Wo, computes its heads' attention and a
partial output product y_c = hc @ Wo[:, c-slice].T, and the host all-reduces
the 8 partials (the "row-shard W_o and all-reduce" variant, with the
all-reduce in the gather step).

Per-core Bass/Tile kernel. The program is emitted as ONE flattened loop over
the 256 global (strip, sk-tile) score tiles; engines execute their streams
in order, so emission order is the software pipeline:
  - slot g: exp(g) on ACT, QK(g+1) lookahead, PV(g-8) (a uniform 8-slot lag
    decouples the exp spine from the DMA-paced arrival of the V strips during
    strip 0), then one dripped projection/epilogue thunk.
  - Projections qhT/khT [128,S] (head A on partitions 0-63, head B on 64-127)
    and vh tiles [sk, dv]+ones-column (PE-transposed). All K/V x-strips are
    DMA'd in arrival-priority order (K two strips ahead of V) through a
    slot-paced pool; thunks are scheduled at the slot where their data lands.
  - QK: scores^T for both heads concurrently (PE row groups at partitions
    0/64); one Exp over the [128,1024] PSUM pair (scale=1/8, no max
    subtraction — scores are O(1)); PV accumulates out^T[dv+1, sq] with a
    ones column yielding the softmax denominator in row 64.
  - Per-strip epilogue, drip-fed into the NEXT strip's slots: approx
    reciprocals straight from the two PSUM denominator rows (no partition
    moves), per-head K=1 ones-matmul broadcasts across 64 partitions at the
    natural base partition (DVE cannot cross partitions), head B's raw
    output DMA-shifted under head A early and normalized in place, then
    single K=128 output-projection matmuls.

Matmul operands are bf16 (full PE rate). PSUM accumulation is fp32; exp
input is exact fp32; denominators stay fp32 into the approx reciprocal.
PSUM (8 banks): qk 2x[128,1024]=4, pv 3x[128,512]=3, post 1x[128,512]=1.
"""

import contextlib
import sys

if "/opt/trn_rl_repo" not in sys.path:
    sys.path.insert(0, "/opt/trn_rl_repo")

import numpy as np
import ml_dtypes

import concourse.bass as bass
import concourse.mybir as mybir
import concourse.tile as tile
from concourse.masks import make_identity

F32 = mybir.dt.float32
BF16 = mybir.dt.bfloat16
EXP = mybir.ActivationFunctionType.Exp
MULT = mybir.AluOpType.mult

S, D, DK, P, W = 4096, 1024, 64, 128, 512
NS = S // W      # 8 sq strips
NT = S // P      # 32 sk tiles
NDC = D // P     # 8 contraction chunks for projections
TPW = W // P     # sk tiles per strip (4)
SCALE = 0.125    # 1/sqrt(DK)
NCORES = 8
NG = NS * NT // TPW * TPW  # 256 global score tiles
GTOT = NS * NT             # 256
LAG = 8                    # PV emission lag (slots); decouples exp from V DMA


def _split_excess_waits(nc, max_waits=1, max_waits_evsem=2):
    """The walrus build in this container rejects instructions carrying more
    than ~2 sync-wait commands; Tile's exit drain aggregates one wait per live
    semaphore onto single instructions. Split the excess onto preceding NoOps
    on the same engine (engine streams are in-order, so semantics hold)."""
    for fn in nc.m.functions:
        for blk in fn.blocks:
            new_insts = []
            for inst in blk.instructions:
                si = getattr(inst, "sync_info", None)
                lim = (
                    max_waits_evsem
                    if isinstance(inst, mybir.InstEventSemaphore)
                    else max_waits
                )
                if si is not None and si.on_wait and len(si.on_wait) > lim:
                    waits = list(si.on_wait)
                    for w in waits[:-lim]:
                        new_insts.append(
                            mybir.InstNoOp(
                                name=nc.get_next_instruction_name(),
                                engine=inst.engine,
                                bass_nofuse=True,
                                sync_info=mybir.SyncInfo(on_wait=[w], on_update=[]),
                            )
                        )
                    si.on_wait = waits[-lim:]
                new_insts.append(inst)
            blk.instructions = new_insts


def _build_mha(nc: bass.Bass):
    qT = nc.dram_tensor("qT", [D, S], BF16, kind="ExternalInput")
    kT = nc.dram_tensor("kT", [D, S], BF16, kind="ExternalInput")
    vT = nc.dram_tensor("vT", [D, S], BF16, kind="ExternalInput")
    # host-packed [p, o*m] so the weight DMA is one contiguous 2KB row per
    # partition (the (o p) m gather pattern was 256B-element descriptor soup)
    wq = nc.dram_tensor("wq", [P, NDC * P], BF16, kind="ExternalInput")
    wk = nc.dram_tensor("wk", [P, NDC * P], BF16, kind="ExternalInput")
    wv = nc.dram_tensor("wv", [P, NDC * P], BF16, kind="ExternalInput")
    wo = nc.dram_tensor("wo", [P, D], BF16, kind="ExternalInput")
    seld = nc.dram_tensor("seld", [2, 2 * DK], BF16, kind="ExternalInput")
    y = nc.dram_tensor("y", [S, D], BF16, kind="ExternalOutput")

    qT3 = qT.rearrange("(o p) s -> p o s", p=P)
    kT3 = kT.rearrange("(o p) s -> p o s", p=P)
    vT3 = vT.rearrange("(o p) s -> p o s", p=P)
    src3 = {"q": qT3, "k": kT3, "v": vT3}

    with tile.TileContext(nc) as tc, contextlib.ExitStack() as ctx:
        static = ctx.enter_context(tc.tile_pool(name="static", bufs=1))
        kvpool = ctx.enter_context(tc.tile_pool(name="kv", bufs=7))
        qpool = ctx.enter_context(tc.tile_pool(name="qx", bufs=2))
        vtmp = ctx.enter_context(tc.tile_pool(name="vtmp", bufs=2))
        ptp = ctx.enter_context(tc.tile_pool(name="pt", bufs=9))
        dpool = ctx.enter_context(tc.tile_pool(name="den", bufs=2))
        ystage = ctx.enter_context(tc.tile_pool(name="ystage", bufs=3))
        qk_ps = ctx.enter_context(tc.tile_pool(name="qkps", bufs=2, space="PSUM"))
        pv_ps = ctx.enter_context(tc.tile_pool(name="pvps", bufs=3, space="PSUM"))
        post_ps = ctx.enter_context(tc.tile_pool(name="postps", bufs=1, space="PSUM"))

        wq_sb = static.tile([P, NDC, P], BF16, tag="wq")
        wk_sb = static.tile([P, NDC, P], BF16, tag="wk")
        wv_sb = static.tile([P, NDC, P], BF16, tag="wv")
        wo_sb = static.tile([P, D], BF16, tag="wo")
        ident = static.tile([P, P], BF16, tag="ident")
        sel = static.tile([DK + 2, 2 * DK], BF16, tag="sel")
        khT = static.tile([P, S], BF16, tag="khT")
        qhT = static.tile([P, S], BF16, tag="qhT")
        vh = static.tile([P, NT, 2 * DK + 2], BF16, tag="vh")
        hc = static.tile([P, S], BF16, tag="hc")

        # identity FIRST on gpsimd so the PE warm-up is not stuck behind
        # later memsets; sel ones row (norm broadcast lhsT) next
        make_identity(nc, ident[:])
        nc.sync.dma_start(sel[DK : DK + 2, :], seld[:])
        # HAM warm-up: dummy PE work (no DMA dependency) so the projections
        # hit the array already ramped.
        warm = post_ps.tile([P, W], F32, tag="post")
        for _ in range(24):
            nc.tensor.matmul(warm[:, 0:P], ident[:], ident[:], start=True, stop=True)

        # ---- DMA issue helpers (SP queue; issue order = transfer order) ----
        def issue_x_dma(which, j, pool, chunks=1):
            jw = slice(j * W, (j + 1) * W)
            xx = pool.tile([P, NDC, W], BF16, tag=which + "x")
            step = NDC // chunks
            for c0 in range(0, NDC, step):
                nc.sync.dma_start(
                    xx[:, c0 : c0 + step, :], src3[which][:, c0 : c0 + step, jw]
                )
            return xx

        def proj_matmuls(w_sb, xx, dst):
            pp = post_ps.tile([P, W], F32, tag="post")
            for c in range(NDC):
                nc.tensor.matmul(
                    pp[:], w_sb[:, c, :], xx[:, c, :],
                    start=(c == 0), stop=(c == NDC - 1),
                )
            nc.vector.tensor_copy(dst, pp[:])

        def v_transposes(j, vts):
            for i in range(TPW):
                t = j * TPW + i
                ptr = post_ps.tile([P, P], BF16, tag="post")
                nc.tensor.transpose(ptr[:], vts[:, i * P : (i + 1) * P], ident[:])
                # ptr rows = sk; cols 0:64 head A dv, 64:128 head B dv
                nc.vector.tensor_copy(vh[:, t, 0:DK], ptr[:, 0:DK])
                nc.vector.tensor_copy(
                    vh[:, t, DK + 1 : 2 * DK + 1], ptr[:, DK : 2 * DK]
                )
                nc.gpsimd.memset(vh[:, t, DK : DK + 1], 1.0)
                nc.gpsimd.memset(vh[:, t, 2 * DK + 1 : 2 * DK + 2], 1.0)

        # ---- prologue: strip 0 at chunk granularity ----
        nc.sync.dma_start(wq_sb[:], wq.rearrange("p (o m) -> p o m", o=NDC))
        nc.sync.dma_start(wk_sb[:], wk.rearrange("p (o m) -> p o m", o=NDC))
        xq0 = issue_x_dma("q", 0, qpool, chunks=4)
        xk0 = issue_x_dma("k", 0, kvpool, chunks=4)
        proj_matmuls(wq_sb, xq0, qhT[:, 0:W])
        proj_matmuls(wk_sb, xk0, khT[:, 0:W])

        qkq = []
        next_qk = [0]

        def push_qk():
            g = next_qk[0]
            if g >= GTOT:
                return
            s2, t2 = divmod(g, NT)
            scw = slice(s2 * W, (s2 + 1) * W)
            qk = qk_ps.tile([P, 2 * W], F32, tag="qk")
            nc.tensor.matmul(
                qk[0:P, 0:W],
                khT[0:DK, t2 * P : (t2 + 1) * P], qhT[0:DK, scw],
                start=True, stop=True,
            )
            nc.tensor.matmul(
                qk[0:P, W : 2 * W],
                khT[DK:P, t2 * P : (t2 + 1) * P], qhT[DK:P, scw],
                start=True, stop=True,
            )
            qkq.append(qk)
            next_qk[0] += 1

        push_qk()  # QK(0,0) right after the k-strip-0 projection

        nc.sync.dma_start(wv_sb[:], wv.rearrange("p (o m) -> p o m", o=NDC))
        xv0 = issue_x_dma("v", 0, kvpool, chunks=4)
        vts0 = vtmp.tile([P, W], BF16, tag="vts")
        proj_matmuls(wv_sb, xv0, vts0[:])
        v_transposes(0, vts0)
        nc.sync.dma_start(wo_sb[:], wo[:])

        # K/V x-strips in arrival-priority order (K ~2 strips ahead of V);
        # the 8-slot kv pool paces transfers against consumption via WARs.
        xt = {}
        for which, j in [
            ("k", 1), ("k", 2), ("v", 1), ("k", 3), ("v", 2), ("k", 4),
            ("v", 3),
        ]:
            xt[(which, j)] = issue_x_dma(which, j, kvpool)
        xt[("q", 1)] = issue_x_dma("q", 1, qpool)
        for which, j in [
            ("k", 5), ("v", 4), ("k", 6), ("k", 7), ("v", 5), ("v", 6),
            ("v", 7),
        ]:
            xt[(which, j)] = issue_x_dma(which, j, kvpool)
        xt[("q", 2)] = issue_x_dma("q", 2, qpool)

        # ---- dripped compute thunks, scheduled at the slot their data lands
        def k_th(j):
            return lambda: proj_matmuls(
                wk_sb, xt[("k", j)], khT[:, j * W : (j + 1) * W]
            )

        def q_th(j):
            return lambda: proj_matmuls(
                wq_sb, xt[("q", j)], qhT[:, j * W : (j + 1) * W]
            )

        vts_tiles = {}

        def v_th(j):
            def _th():
                vts_tiles[j] = vtmp.tile([P, W], BF16, tag="vts", name="vts")
                proj_matmuls(wv_sb, xt[("v", j)], vts_tiles[j][:])
            return _th

        def trs_th(j):
            return lambda: v_transposes(j, vts_tiles[j])

        drip = {
            2: k_th(1), 6: k_th(2), 8: v_th(1), 9: trs_th(1), 10: k_th(3),
            12: v_th(2), 13: trs_th(2), 14: k_th(4), 16: v_th(3),
            17: trs_th(3), 18: k_th(5), 20: v_th(4), 21: trs_th(4),
            22: k_th(6), 24: k_th(7), 26: v_th(5), 27: trs_th(5),
            28: q_th(1), 29: v_th(6), 30: trs_th(6), 31: v_th(7),
            32: trs_th(7),
        }

        # ---- epilogue (per strip, dripped into the next strip's slots) ----
        def make_epilogue(s, rec):
            cw = slice(s * W, (s + 1) * W)
            thunks = []

            def norm_a():
                bc = post_ps.tile([P, W], F32, tag="post")
                nc.tensor.matmul(
                    bc[0:DK, :], sel[DK : DK + 2, 0:DK], rec,
                    start=True, stop=True,
                )
                nc.vector.tensor_tensor(
                    hc[0:DK, cw], hc[0:DK, cw], bc[0:DK, :], op=MULT
                )

            def norm_b():
                bc = post_ps.tile([P, W], F32, tag="post")
                nc.tensor.matmul(
                    bc[DK:P, :], sel[DK : DK + 2, DK : 2 * DK], rec,
                    start=True, stop=True,
                )
                nc.vector.tensor_tensor(
                    hc[DK:P, cw], hc[DK:P, cw], bc[DK:P, :], op=MULT
                )

            thunks.append(norm_a)
            thunks.append(norm_b)

            def proj_out(i, oh):
                def _th():
                    sq = s * TPW + i
                    if s == NS - 1:
                        # tail: the qk banks are dead — free slots beat
                        # WAR-serializing through the shared post pool
                        py = qk_ps.tile([P, W], F32, tag="qk")
                    else:
                        py = post_ps.tile([P, W], F32, tag="post")
                    nc.tensor.matmul(
                        py[:],
                        hc[:, sq * P : (sq + 1) * P],
                        wo_sb[:, oh * W : (oh + 1) * W],
                        start=True, stop=True,
                    )
                    ys = ystage.tile([P, W], BF16, tag="ys")
                    # final strip: exp stream is over, ScalarE is idle — let
                    # it take half the PSUM evacuations
                    if s == NS - 1 and (i + oh) % 2 == 1:
                        nc.scalar.copy(ys[:], py[:])
                    else:
                        nc.vector.tensor_copy(ys[:], py[:])
                    nc.sync.dma_start(
                        y[sq * P : (sq + 1) * P, oh * W : (oh + 1) * W], ys[:]
                    )
                return _th

            for i in range(TPW):
                for oh in range(2):
                    thunks.append(proj_out(i, oh))
            return thunks

        # ---- flattened main loop over global score tiles ----
        pvt = {}      # strip -> (pva, pvb)
        ptq = []      # exp'd-but-not-PV'd probability tiles
        pending = []  # epilogue thunks dripping

        def emit_pv(g):
            s2, t2 = divmod(g, NT)
            if t2 == 0:
                pva_new = pv_ps.tile([P, W], F32, tag="pv")
                pvb_new = pv_ps.tile([P, W], F32, tag="pv")
                pvt[s2] = (pva_new, pvb_new)
            pva, pvb = pvt[s2]
            pt = ptq.pop(0)
            nc.tensor.matmul(
                pva[0 : DK + 1, :],
                vh[:, t2, 0 : DK + 1], pt[:, 0:W],
                start=(t2 == 0), stop=(t2 == NT - 1),
            )
            nc.tensor.matmul(
                pvb[0 : DK + 1, :],
                vh[:, t2, DK + 1 : 2 * DK + 2], pt[:, W : 2 * W],
                start=(t2 == 0), stop=(t2 == NT - 1),
            )

        def emit_boundary(s2):
            """Strip s2's PV accumulation just ended: evacuate, reciprocal,
            and queue the normalize/out-proj epilogue."""
            nonlocal pending
            cw = slice(s2 * W, (s2 + 1) * W)
            pva, pvb = pvt.pop(s2)
            last = s2 == NS - 1
            # head B raw output shifted under head A immediately (normalized
            # in place later, while the DMA is in flight)
            tmpb = ystage.tile([DK, W], BF16, tag="tmpb", name="tmpb")
            if last:
                nc.scalar.copy(tmpb[0:DK, :], pvb[0:DK, :])
                nc.sync.dma_start(hc[DK:P, cw], tmpb[0:DK, :])
                nc.scalar.copy(hc[0:DK, cw], pva[0:DK, :])
            else:
                nc.vector.tensor_copy(tmpb[0:DK, :], pvb[0:DK, :])
                nc.sync.dma_start(hc[DK:P, cw], tmpb[0:DK, :])
                nc.vector.tensor_copy(hc[0:DK, cw], pva[0:DK, :])
            rs = dpool.tile([DK + 2, W], F32, tag="rs")
            rbt = dpool.tile([DK + 1, W], F32, tag="rbt")
            nc.vector.tensor_copy(rs[DK : DK + 1, :], pva[DK : DK + 1, :])
            nc.vector.tensor_copy(rbt[DK : DK + 1, :], pvb[DK : DK + 1, :])
            nc.sync.dma_start(rs[DK + 1 : DK + 2, :], rbt[DK : DK + 1, :])
            rec = dpool.tile([DK + 2, W], BF16, tag="rec")
            with nc.allow_low_precision(
                reason="bf16 softmax denominators feed a bf16 matmul bcast"
            ):
                nc.vector.reciprocal(rec[DK : DK + 2, :], rs[DK : DK + 2, :])
            assert not pending, f"epilogue {s2 - 1} not drained"
            pending = make_epilogue(s2, rec[DK : DK + 2, :])
            # drip the q projection consumed by strip s2+2's QKs, and
            # prefetch the q x-strip the NEXT boundary's q-drip will need
            if s2 + 2 <= NS - 1:
                pending.insert(0, q_th(s2 + 2))
            if s2 + 3 <= NS - 1:
                xt[("q", s2 + 3)] = issue_x_dma("q", s2 + 3, qpool)

        for g in range(GTOT):
            # exp spine first: consume QK(g), then emit QK(g+1) lookahead
            qk = qkq.pop(0)
            pt = ptp.tile([P, 2 * W], BF16, tag="pt")
            nc.scalar.activation(pt[:], qk[:], EXP, scale=SCALE)
            ptq.append(pt)
            push_qk()
            if g >= LAG:
                emit_pv(g - LAG)
                if (g - LAG) % NT == NT - 1:
                    emit_boundary((g - LAG) // NT)
            th = drip.pop(g, None)
            if th is not None:
                th()
            elif pending and g % NT >= LAG and g % NT % 2 == 0:
                pending.pop(0)()

        # drain: the last LAG PVs chase the final exps, then the tail epilogue
        for g in range(GTOT, GTOT + LAG):
            emit_pv(g - LAG)
            if (g - LAG) % NT == NT - 1:
                emit_boundary((g - LAG) // NT)
        for th in pending:
            th()
        pending = []
    return nc


def _make_core_inputs(q, k, v, Wq, Wk, Wv, Wo, core, cache):
    bf = ml_dtypes.bfloat16

    def pack_w(Wx):
        # [D, 128] -> [p, o*m]: row d = o*128+p of the (o p) layout
        return np.ascontiguousarray(
            Wx.reshape(NDC, P, P).transpose(1, 0, 2).reshape(P, NDC * P)
        ).astype(bf)

    if "qT" not in cache:
        cache["qT"] = np.ascontiguousarray(q.T).astype(bf)
        cache["kT"] = np.ascontiguousarray(k.T).astype(bf)
        cache["vT"] = np.ascontiguousarray(v.T).astype(bf)
    h0, h1 = 2 * core, 2 * core + 1
    return {
        "qT": cache["qT"],
        "kT": cache["kT"],
        "vT": cache["vT"],
        "wq": pack_w(np.concatenate([Wq[h0], Wq[h1]], axis=1)),
        "wk": pack_w(np.concatenate([Wk[h0], Wk[h1]], axis=1)),
        "wv": pack_w(np.concatenate([Wv[h0], Wv[h1]], axis=1)),
        "wo": np.ascontiguousarray(Wo[:, P * core : P * (core + 1)].T).astype(bf),
        "seld": np.kron(
            np.eye(2, dtype=np.float32), np.ones((1, DK), np.float32)
        ).astype(bf),
    }


_NC = None
last_results = None  # BassKernelResults of the most recent run (for profiling)


def _get_nc():
    global _NC
    if _NC is None:
        nc = bass.Bass("TRN2", target_bir_lowering=False, debug=False)
        _build_mha(nc)
        _split_excess_waits(nc)
        _NC = nc
    return _NC


def kernel(q, k, v, Wq, Wk, Wv, Wo, **run_kwargs):
    """Full-input MHA forward. Shards over 8 NeuronCores (2 heads each),
    runs the Bass kernel, and all-reduces the per-core partial outputs."""
    from concourse.bass_utils import run_bass_kernel_spmd

    global last_results
    q = np.asarray(q, np.float32)
    k = np.asarray(k, np.float32)
    v = np.asarray(v, np.float32)
    Wq = np.asarray(Wq, np.float32)
    Wk = np.asarray(Wk, np.float32)
    Wv = np.asarray(Wv, np.float32)
    Wo = np.asarray(Wo, np.float32)

    nc = _get_nc()
    cache = {}
    in_maps = [
        _make_core_inputs(q, k, v, Wq, Wk, Wv, Wo, c, cache) for c in range(NCORES)
    ]
    res = run_bass_kernel_spmd(
        nc, in_maps, core_ids=list(range(NCORES)), **run_kwargs
    )
    last_results = res
    y = res.results[0]["y"].astype(np.float32)
    for c in range(1, NCORES):
        y += res.results[c]["y"]
    return y
